# revision 3
# baseline (speedup 1.0000x reference)
"""Trainium2 Bass kernel for BinaryConv2dSkip1x1.

Reference computation (per batch image, B=8 sharded across 8 cores):
    xb   = sign(x + move0_bias)                      # binarize, values in {-1,0,+1}
    raw  = conv3x3(xb, sign(weight), pad=1)          # integer counts (exact in bf16)
    v    = sf * raw + pr_bias0                       # sf = mean|weight| per out-ch
    p    = max(v, a*v)                               # PReLU (a in [0,1])
    out  = p + pr_bias1 + conv1x1(x, skip_w) + skip_b

Identities used:
    prelu(v) + c == max(v + c, a*v + (1-a)*c + c - c) ... concretely:
      v' = sf*raw + (b0 + c);  av' = a*v' + (1-a)*c;  max(v', av') == max(v, a*v) + c
    with c = pr_bias1 + skip_b folded per-channel.

Layout per core:
    SBUF partitions 0:64   = channels x image rows   0..127  ("low")
    SBUF partitions 64:128 = channels x image rows 128..255  ("high")
    8 chunks of 16 rows per group; per chunk 8 psum tiles [128, 2, 256]
    (partitions 0:64 <- row pair 2j, partitions 64:128 <- row pair 8+2j)
    4-way PE quadrant packing via tile_position in {0,64} x {0,64}.
"""

import sys
import os

for _p in ("/opt/trn_rl_repo",):
    if _p not in sys.path:
        sys.path.insert(0, _p)

from contextlib import ExitStack

import numpy as np
import ml_dtypes

import concourse.bass as bass
import concourse.tile as tile
from concourse import bacc, mybir

F32 = mybir.dt.float32
BF16 = mybir.dt.bfloat16
AF = mybir.ActivationFunctionType
ALU = mybir.AluOpType

C = 64          # channels (in == out)
H = 256         # image height
W = 256         # image width
NCHUNK = 8      # chunks per partition-group
RCHUNK = 16     # image rows per chunk per group
NCORES = 8


def emit_pipeline(nc, tc, ctx, x_d, out_d, wmain_t, wskip_t, consts_t, pools):
    xpool, xbpool, pmain, pskip, vpool, avpool, mpool, outpool = pools

    move0 = consts_t[:, 0:1]
    sf = consts_t[:, 1:2]
    b0c = consts_t[:, 2:3]
    a_ap = consts_t[:, 3:4]
    c2 = consts_t[:, 4:5]

    for k in range(NCHUNK):
        # ---- load x chunk: 18 halo'd rows per partition group ----
        x_t = xpool.tile([128, 18, W], F32)
        # low group: rows 16k-1 .. 16k+16 ; high group: rows 128+16k-1 .. +16
        if k == 0:
            nc.gpsimd.memset(x_t[0:64, 0, :], 0.0)
            nc.sync.dma_start(x_t[0:64, 1:18, :], x_d[:, 0:17, :])
        else:
            r0 = RCHUNK * k - 1
            nc.sync.dma_start(x_t[0:64, :, :], x_d[:, r0:r0 + 18, :])
        if k == NCHUNK - 1:
            r0 = 128 + RCHUNK * k - 1
            nc.sync.dma_start(x_t[64:128, 0:17, :], x_d[:, r0:r0 + 17, :])
            nc.gpsimd.memset(x_t[64:128, 17, :], 0.0)
        else:
            r0 = 128 + RCHUNK * k - 1
            nc.sync.dma_start(x_t[64:128, :, :], x_d[:, r0:r0 + 18, :])

        # ---- binarize: xb = sign(x + move0), bf16, with zero col pads ----
        xb_t = xbpool.tile([128, 18, W + 2], BF16)
        nc.gpsimd.memset(xb_t[:, :, 0], 0.0)
        nc.gpsimd.memset(xb_t[:, :, W + 1], 0.0)
        nc.scalar.activation(xb_t[:, :, 1:W + 1], x_t[:, :, :], AF.Sign, bias=move0)
        if k == 0:
            nc.vector.memset(xb_t[0:64, 0, 1:W + 1], 0.0)      # top pad row
        if k == NCHUNK - 1:
            nc.vector.memset(xb_t[64:128, 17, 1:W + 1], 0.0)   # bottom pad row

        # ---- per chunk: 8 iterations over (group, row-pair) ----
        outstg = [None, None]
        for j in range(8):
            g = j % 2            # 0: low rows (xb parts 0:64), 1: high rows
            jj = j // 2          # row-pair index 0..3
            gp = 64 * g

            if outstg[g] is None:
                outstg[g] = outpool.tile([128, 8, W], F32, name="outstg",
                                         tag="outstg")

            # main binarized conv: 9 positions x 2 output col-groups
            pm = pmain.tile([128, 2, W], F32)
            for ky in range(3):
                for kx in range(3):
                    pos = ky * 3 + kx
                    lhsT = wmain_t[gp:gp + 64, 64 * pos:64 * pos + 64]
                    st = pos == 0
                    sp = pos == 8
                    nc.tensor.matmul(
                        pm[0:64, :, :], lhsT,
                        xb_t[gp:gp + 64, 2 * jj + ky:2 * jj + ky + 2, kx:kx + W],
                        start=st, stop=sp, tile_position=(gp, 0),
                        skip_group_check=True,
                    )
                    nc.tensor.matmul(
                        pm[64:128, :, :], lhsT,
                        xb_t[gp:gp + 64, 8 + 2 * jj + ky:8 + 2 * jj + ky + 2, kx:kx + W],
                        start=st, stop=sp, tile_position=(gp, 64),
                        skip_group_check=True,
                    )

            # 1x1 skip conv in fp32 on the raw x tile
            ps = pskip.tile([128, 2, W], F32)
            nc.tensor.matmul(
                ps[0:64, :, :], wskip_t[gp:gp + 64, :],
                x_t[gp:gp + 64, 1 + 2 * jj:3 + 2 * jj, :],
                start=True, stop=True, tile_position=(gp, 0),
                skip_group_check=True,
            )
            nc.tensor.matmul(
                ps[64:128, :, :], wskip_t[gp:gp + 64, :],
                x_t[gp:gp + 64, 9 + 2 * jj:11 + 2 * jj, :],
                start=True, stop=True, tile_position=(gp, 64),
                skip_group_check=True,
            )

            # v' = sf*raw + (b0 + c)  [ACT from PSUM]
            v_t = vpool.tile([128, 2, W], BF16)
            nc.scalar.activation(v_t[:], pm[:], AF.Identity, bias=b0c, scale=sf)
            # av' = a*v' + (1-a)*c
            av_t = avpool.tile([128, 2, W], BF16)
            nc.vector.tensor_scalar(av_t[:], v_t[:], a_ap, c2, ALU.mult, ALU.add)
            # m = max(v', av') == prelu(v) + c
            m_t = mpool.tile([128, 2, W], BF16)
            nc.vector.tensor_tensor(m_t[:], v_t[:], av_t[:], ALU.max)
            # out = m + skip
            nc.vector.tensor_tensor(
                outstg[g][:, 2 * jj:2 * jj + 2, :], m_t[:], ps[:], ALU.add)

        # ---- store chunk outputs ----
        for g in range(2):
            rowbase = 128 * g + RCHUNK * k
            nc.gpsimd.dma_start(out_d[:, rowbase:rowbase + 8, :], outstg[g][0:64, :, :])
            nc.gpsimd.dma_start(out_d[:, rowbase + 8:rowbase + 16, :], outstg[g][64:128, :, :])


def build(reps=1):
    nc = bacc.Bacc("TRN2", target_bir_lowering=False, debug=False)

    x_d = nc.dram_tensor("x", [C, H, W], F32, kind="ExternalInput")
    wmain_d = nc.dram_tensor("wmain", [128, 576], BF16, kind="ExternalInput")
    wskip_d = nc.dram_tensor("wskip", [128, 64], F32, kind="ExternalInput")
    consts_d = nc.dram_tensor("consts", [128, 5], F32, kind="ExternalInput")
    out_d = nc.dram_tensor("out", [C, H, W], F32, kind="ExternalOutput")

    with tile.TileContext(nc) as tc:
        with ExitStack() as ctx:
            wpool = ctx.enter_context(tc.tile_pool(name="wpool", bufs=1))
            xpool = ctx.enter_context(tc.tile_pool(name="xpool", bufs=2))
            xbpool = ctx.enter_context(tc.tile_pool(name="xbpool", bufs=2))
            pmain = ctx.enter_context(tc.tile_pool(name="pmain", bufs=4, space="PSUM"))
            pskip = ctx.enter_context(tc.tile_pool(name="pskip", bufs=4, space="PSUM"))
            vpool = ctx.enter_context(tc.tile_pool(name="vpool", bufs=3))
            avpool = ctx.enter_context(tc.tile_pool(name="avpool", bufs=3))
            mpool = ctx.enter_context(tc.tile_pool(name="mpool", bufs=3))
            outpool = ctx.enter_context(tc.tile_pool(name="outpool", bufs=4))

            wmain_t = wpool.tile([128, 576], BF16)
            nc.sync.dma_start(wmain_t[:], wmain_d[:])
            wskip_t = wpool.tile([128, 64], F32)
            nc.sync.dma_start(wskip_t[:], wskip_d[:])
            consts_t = wpool.tile([128, 5], F32)
            nc.sync.dma_start(consts_t[:], consts_d[:])

            pools = (xpool, xbpool, pmain, pskip, vpool, avpool, mpool, outpool)
            for _ in range(reps):
                emit_pipeline(nc, tc, ctx, x_d, out_d, wmain_t, wskip_t,
                              consts_t, pools)

    nc.compile()
    return nc


def host_prep(weight, move0_bias, pr_bias0, prelu_w, pr_bias1, skip_w, skip_b):
    """Precompute weight/constant tensors shared by all cores."""
    weight = np.asarray(weight, np.float32)
    sgnw = np.sign(weight)                                   # [o, i, ky, kx]
    wmain = sgnw.transpose(1, 2, 3, 0).reshape(C, 9 * C)     # [i, pos*64+o]
    wmain = np.concatenate([wmain, wmain], axis=0).astype(ml_dtypes.bfloat16)

    wskip = np.asarray(skip_w, np.float32)[:, :, 0, 0].T     # [i, o]
    wskip = np.concatenate([wskip, wskip], axis=0).astype(np.float32)

    sf = np.abs(weight).mean(axis=(1, 2, 3))                 # [o]
    b0 = np.asarray(pr_bias0, np.float32).reshape(C)
    b1 = np.asarray(pr_bias1, np.float32).reshape(C)
    sb = np.asarray(skip_b, np.float32).reshape(C)
    a = np.asarray(prelu_w, np.float32).reshape(C)
    m0 = np.asarray(move0_bias, np.float32).reshape(C)
    c = b1 + sb
    consts = np.stack([m0, sf, b0 + c, a, (1.0 - a) * c], axis=1)  # [64, 5]
    consts = np.concatenate([consts, consts], axis=0).astype(np.float32)
    return wmain, wskip, consts


_NC_CACHE = {}


def _get_nc(reps=1):
    if reps not in _NC_CACHE:
        _NC_CACHE[reps] = build(reps)
    return _NC_CACHE[reps]


def kernel(x, move0_bias, weight, pr_bias0, prelu_w, pr_bias1, skip_w, skip_b,
           _reps=1, _return_raw=False):
    from concourse.bass_utils import run_bass_kernel_spmd

    x = np.asarray(x, np.float32)
    B = x.shape[0]
    assert x.shape == (B, C, H, W)
    wmain, wskip, consts = host_prep(weight, move0_bias, pr_bias0, prelu_w,
                                     pr_bias1, skip_w, skip_b)

    nc = _get_nc(_reps)
    in_maps = [
        {"x": np.ascontiguousarray(x[b]), "wmain": wmain, "wskip": wskip,
         "consts": consts}
        for b in range(B)
    ]
    res = run_bass_kernel_spmd(nc, in_maps, core_ids=list(range(B)))
    out = np.stack([res.results[b]["out"] for b in range(B)], axis=0)
    if _return_raw:
        return out, res
    return out
